# revision 7
# baseline (speedup 1.0000x reference)
"""Trainium2 Bass kernel for the LeNet-C3-style binarized 16-branch 5x5 conv.

Problem: input [64, 6, 256, 256] f32 -> output [64, 16, 252, 252] f32.
Each of 16 branches g convolves a channel subset S_g (classic LeNet C3
connectivity) with a binarized 5x5 kernel whose taps are all +-alpha_g.

Strategy (pure data parallel over 8 NeuronCores, 8 images per core):
  - Host: binarize weights -> sign tensor (+-1/0, exact in bf16) and per-branch
    scale alpha_g (folded into the PSUM evacuation, so the matmul itself
    carries no weight quantization error).
  - Device, per image: view the image as 32 row-chunks of 8 rows. SBUF X-tile
    [72, 32, 256]: partition (r*6+c), chunk t holds input row 8t+r of channel
    c (12-row window = 8 output rows + 4 halo; 1.5x HBM read amplification).
    For each pair of chunks, 5 accumulating matmuls (one per kw tap, rhs
    shifted by kw in the width dim) against a stationary [72, 128] sign matrix
    produce PSUM[(i*16+g), (t,j)] = conv for 16 output rows x 16 channels x
    252 columns. DVE/ACT evacuate PSUM -> SBUF applying the per-partition
    alpha_g scale; strided DMAs write half-images back to HBM.
"""

import numpy as np
import ml_dtypes

import concourse.bass as bass
import concourse.mybir as mybir
import concourse.tile as tile
from concourse import bacc, bass_utils

# LeNet C3 connectivity
FM = [[0, 1, 2], [1, 2, 3], [2, 3, 4], [3, 4, 5], [0, 4, 5], [0, 1, 5],
      [0, 1, 2, 3], [1, 2, 3, 4], [2, 3, 4, 5], [0, 3, 4, 5], [0, 1, 4, 5],
      [0, 1, 2, 5], [0, 1, 3, 4], [1, 2, 4, 5], [0, 2, 3, 5],
      [0, 1, 2, 3, 4, 5]]

N_CORES = 8
B_PER_CORE = 8
H = W = 256
HO = WO = 252
IMG = 6 * H * W            # 393216 elements per input image
OIMG = 16 * HO * WO        # 1016064 elements per output image
XPAD = 1024                # flat-input tail padding (chunk-31 halo overrun)
# moving/stationary matmul dtype: "bf16" (1 cyc/row, input rounded to bf16),
# "fp32" (exact, 4 cyc/row), "fp32r" (1 cyc/row at N>=256, HW-reduced fp32)
MM_DTYPE = "fp32r"

_CACHED = {}


def _binarize_np(w):
    saidas = float(w.shape[1] * w.shape[2] * w.shape[3])
    mw = w - np.mean(w, axis=1, keepdims=True, dtype=np.float32)
    cw = np.clip(mw, -1.0, 1.0)
    alpha = np.sum(np.abs(cw), axis=(1, 2, 3)) / saidas
    return np.sign(cw).astype(np.float32), alpha.astype(np.float32)


def _pack_weights(W3, W4, W6):
    """sign_dense [16,6,5,5] in {-1,0,1}; alpha [16]."""
    sign_dense = np.zeros((16, 6, 5, 5), np.float32)
    alpha = np.zeros(16, np.float32)
    for stack, base in ((W3, 0), (W4, 6), (W6, 15)):
        s, a = _binarize_np(np.asarray(stack, np.float32))
        for gi in range(s.shape[0]):
            g = base + gi
            for ci, c in enumerate(FM[g]):
                sign_dense[g, c] = s[gi, ci]
            alpha[g] = a[gi]
    return sign_dense, alpha


def _build_lhsT(sign_dense):
    """lhsT [5(kw), 72(r*6+c), 128(i*16+g)]."""
    lhsT = np.zeros((5, 72, 128), np.float32)
    for kw in range(5):
        for i in range(8):
            for kh in range(5):
                r = i + kh
                for c in range(6):
                    lhsT[kw, r * 6 + c, i * 16:(i + 1) * 16] = \
                        sign_dense[:, c, kh, kw]
    return lhsT


def _mm_np_dtype():
    return {"bf16": ml_dtypes.bfloat16,
            "fp32": np.float32,
            "fp32r": np.float32}[MM_DTYPE]


def _mm_bir_dtype():
    return {"bf16": mybir.dt.bfloat16,
            "fp32": mybir.dt.float32,
            "fp32r": mybir.dt.float32r}[MM_DTYPE]


def _emit(tc, x, w, alpha, out):
    """Emit the per-core program.

    x:     dram [B_PER_CORE*IMG + XPAD] f32 (flat, zero-padded)
    w:     dram [72, 5, 128] mm-dtype (sign weights per kw tap)
    alpha: dram [128, 1] f32 (alpha[g] tiled over i)
    out:   dram [B_PER_CORE, 16, 252, 252] f32
    """
    nc = tc.nc
    mdt = _mm_bir_dtype()
    f32 = mybir.dt.float32
    cast_in = MM_DTYPE == "bf16"
    x_t = x.tensor
    out_t = out.tensor

    with (
        tc.tile_pool(name="singles", bufs=1) as singles,
        tc.tile_pool(name="xp", bufs=2) as xp,
        tc.tile_pool(name="op", bufs=2) as op,
        tc.tile_pool(name="pp", bufs=6, space="PSUM") as pp,
    ):
        w_tile = singles.tile([72, 5, 128], mdt)
        nc.sync.dma_start(out=w_tile, in_=w)
        a_tile = singles.tile([128, 1], f32)
        nc.sync.dma_start(out=a_tile, in_=alpha)

        for b in range(B_PER_CORE):
            xt = xp.tile([72, 32, 256], mdt)
            # one DMA per image: partition r*6+c, chunk t <- x[b, c, 8t+r, :]
            # (c,t) merge exactly into one dim of stride 8 rows; chunk 31's
            # r>=8 lanes read neighbor-channel rows / the zero tail padding --
            # they only feed output rows that are never written back.
            src = bass.AP(
                tensor=x_t, offset=b * IMG,
                ap=[[W, 12], [8 * W, 6 * 32], [1, W]])
            if cast_in:
                nc.gpsimd.dma_start(out=xt, in_=src)
            else:
                nc.sync.dma_start(out=xt, in_=src)

            for h in range(2):
                ot = op.tile([128, 16, WO], f32)
                for u in range(8):
                    t0 = 16 * h + 2 * u
                    ps = pp.tile([128, 2, WO], f32)
                    for kw in range(5):
                        nc.tensor.matmul(
                            ps,
                            lhsT=w_tile[:, kw, :],
                            rhs=xt[:, t0:t0 + 2, kw:kw + WO],
                            start=(kw == 0), stop=(kw == 4))
                    dst = ot[:, 2 * u:2 * u + 2, :]
                    if u % 2 == 0:
                        nc.vector.tensor_scalar_mul(dst, ps, a_tile)
                    else:
                        nc.scalar.activation(
                            dst, ps, mybir.ActivationFunctionType.Copy,
                            scale=a_tile)
                # write back: rows 128h + {8c+i}, one DMA per i (16
                # contiguous partitions each), 15 chunks in h=1 (chunk 31's
                # valid 4 rows go in the tail DMA below)
                nch = 16 if h == 0 else 15
                for i in range(8):
                    dst_ap = bass.AP(
                        tensor=out_t,
                        offset=b * OIMG + (128 * h + i) * WO,
                        ap=[[HO * WO, 16], [8 * WO, nch], [1, WO]])
                    nc.sync.dma_start(out=dst_ap,
                                      in_=ot[i * 16:(i + 1) * 16, 0:nch, :])
                if h == 1:
                    # out rows 248..251 (chunk 31, i in [0,4))
                    dst_ap = bass.AP(
                        tensor=out_t, offset=b * OIMG + 248 * WO,
                        ap=[[WO, 4], [HO * WO, 16], [1, WO]])
                    nc.sync.dma_start(out=dst_ap, in_=ot[0:64, 15, :])


def _build_nc():
    nc = bacc.Bacc("TRN2", target_bir_lowering=False, debug=False,
                   num_devices=N_CORES)
    f32 = mybir.dt.float32
    # bf16 mode casts during the (SWDGE) input DMA, so dram x stays f32;
    # fp32/fp32r modes DMA via HWDGE with no cast, so dtypes must match.
    x_dt = f32 if MM_DTYPE == "bf16" else _mm_bir_dtype()
    x = nc.dram_tensor("x", [B_PER_CORE * IMG + XPAD], x_dt,
                       kind="ExternalInput").ap()
    w = nc.dram_tensor("w", [72, 5, 128], _mm_bir_dtype(),
                       kind="ExternalInput").ap()
    alpha = nc.dram_tensor("alpha", [128, 1], f32, kind="ExternalInput").ap()
    out = nc.dram_tensor("out", [B_PER_CORE, 16, HO, WO], f32,
                         kind="ExternalOutput").ap()
    with tile.TileContext(nc) as tc:
        _emit(tc, x, w, alpha, out)
    nc.compile()
    return nc


def _get_nc():
    if "nc" not in _CACHED:
        _CACHED["nc"] = _build_nc()
    return _CACHED["nc"]


def _make_in_maps(input, W3, W4, W6):
    sign_dense, alpha16 = _pack_weights(W3, W4, W6)
    lhsT = _build_lhsT(sign_dense)                       # [5, 72, 128]
    w_host = np.ascontiguousarray(
        lhsT.transpose(1, 0, 2)).astype(_mm_np_dtype())  # [72, 5, 128]
    alpha_host = np.tile(alpha16, 8).reshape(128, 1).astype(np.float32)
    inp = np.asarray(input, np.float32)
    pad = np.zeros(XPAD, np.float32)
    in_maps = []
    for c in range(N_CORES):
        xc = inp[c * B_PER_CORE:(c + 1) * B_PER_CORE].ravel()
        in_maps.append({
            "x": np.concatenate([xc, pad]),
            "w": w_host,
            "alpha": alpha_host,
        })
    return in_maps


def run(input, W3, W4, W6, trace=False):
    """Run the kernel; returns (full_output, BassKernelResults)."""
    nc = _get_nc()
    in_maps = _make_in_maps(input, W3, W4, W6)
    res = bass_utils.run_bass_kernel_spmd(
        nc, in_maps, core_ids=list(range(N_CORES)), trace=trace)
    out = np.concatenate([np.asarray(r["out"]) for r in res.results], axis=0)
    return out, res


def kernel(input, W3, W4, W6):
    out, _ = run(input, W3, W4, W6)
    return out


# revision 13
# speedup vs baseline: 1.5156x; 1.5156x over previous
"""Trainium2 Bass kernel for the LeNet-C3-style binarized 16-branch 5x5 conv.

Problem: input [64, 6, 256, 256] f32 -> output [64, 16, 252, 252] f32.
Each of 16 branches g convolves a channel subset S_g (classic LeNet C3
connectivity) with a binarized 5x5 kernel whose taps are all +-alpha_g.

Strategy (pure data parallel over 8 NeuronCores, 8 images per core):
  - Host: binarize weights -> sign tensor (+-1/0, exact in bf16) and per-branch
    scale alpha_g (folded into the PSUM evacuation, so the matmul itself
    carries no weight quantization error).
  - Device, per image: view the image as 32 row-chunks of 8 rows. SBUF X-tile
    [72, 32, 256]: partition (r*6+c), chunk t holds input row 8t+r of channel
    c (12-row window = 8 output rows + 4 halo; 1.5x HBM read amplification).
    For each pair of chunks, 5 accumulating matmuls (one per kw tap, rhs
    shifted by kw in the width dim) against a stationary [72, 128] sign matrix
    produce PSUM[(i*16+g), (t,j)] = conv for 16 output rows x 16 channels x
    252 columns. DVE/ACT evacuate PSUM -> SBUF applying the per-partition
    alpha_g scale; strided DMAs write half-images back to HBM.
"""

import numpy as np
import ml_dtypes

import concourse.bass as bass
import concourse.mybir as mybir
import concourse.tile as tile
from concourse import bacc, bass_utils

# LeNet C3 connectivity
FM = [[0, 1, 2], [1, 2, 3], [2, 3, 4], [3, 4, 5], [0, 4, 5], [0, 1, 5],
      [0, 1, 2, 3], [1, 2, 3, 4], [2, 3, 4, 5], [0, 3, 4, 5], [0, 1, 4, 5],
      [0, 1, 2, 5], [0, 1, 3, 4], [1, 2, 4, 5], [0, 2, 3, 5],
      [0, 1, 2, 3, 4, 5]]

N_CORES = 8
B_PER_CORE = 8
H = W = 256
HO = WO = 252
IMG = 6 * H * W            # 393216 elements per input image
OIMG = 16 * HO * WO        # 1016064 elements per output image
XPAD = 1024                # flat-input tail padding (chunk-31 halo overrun)
# moving/stationary matmul dtype: "bf16" (1 cyc/row, input rounded to bf16),
# "fp32" (exact, 4 cyc/row), "fp32r" (1 cyc/row at N>=256, HW-reduced fp32)
MM_DTYPE = "bf16"

_CACHED = {}


def _binarize_np(w):
    saidas = float(w.shape[1] * w.shape[2] * w.shape[3])
    mw = w - np.mean(w, axis=1, keepdims=True, dtype=np.float32)
    cw = np.clip(mw, -1.0, 1.0)
    alpha = np.sum(np.abs(cw), axis=(1, 2, 3)) / saidas
    return np.sign(cw).astype(np.float32), alpha.astype(np.float32)


def _pack_weights(W3, W4, W6):
    """sign_dense [16,6,5,5] in {-1,0,1}; alpha [16]."""
    sign_dense = np.zeros((16, 6, 5, 5), np.float32)
    alpha = np.zeros(16, np.float32)
    for stack, base in ((W3, 0), (W4, 6), (W6, 15)):
        s, a = _binarize_np(np.asarray(stack, np.float32))
        for gi in range(s.shape[0]):
            g = base + gi
            for ci, c in enumerate(FM[g]):
                sign_dense[g, c] = s[gi, ci]
            alpha[g] = a[gi]
    return sign_dense, alpha


def _build_lhsT(sign_dense):
    """lhsT [5(kw), 48(r*6+c; r in [0,8)), 64(i*16+g; i in [0,4))].

    One 4-output-row stream; the same matrix serves both column-tiled
    streams (stream B is the identical structure shifted 4 rows down).
    """
    lhsT = np.zeros((5, 48, 64), np.float32)
    for kw in range(5):
        for i in range(4):
            for kh in range(5):
                r = i + kh
                for c in range(6):
                    lhsT[kw, r * 6 + c, i * 16:(i + 1) * 16] = \
                        sign_dense[:, c, kh, kw]
    return lhsT


def _mm_np_dtype():
    return {"bf16": ml_dtypes.bfloat16,
            "fp32": np.float32,
            "fp32r": np.float32}[MM_DTYPE]


def _mm_bir_dtype():
    return {"bf16": mybir.dt.bfloat16,
            "fp32": mybir.dt.float32,
            "fp32r": mybir.dt.float32r}[MM_DTYPE]


def _emit(tc, x, w, alpha, out):
    """Emit the per-core program.

    x:     dram [B_PER_CORE*IMG + XPAD] mm-dtype (flat, zero-padded)
    w:     dram [112, 5, 64] mm-dtype (sign weights per kw tap; the same
           [48, 64] matrix at partition 0 (stream A) and 64 (stream B))
    alpha: dram [128, 1] f32 (alpha[g] tiled over i)
    out:   dram [B_PER_CORE, 16, 252, 252] f32

    Two column-tiled matmul streams run concurrently in disjoint PE
    quadrants: stream A (array rows 0-47, cols 0-63) computes output rows
    8t..8t+3; stream B (rows 64-111, cols 64-127) rows 8t+4..8t+7. Each
    stream's X block is a full non-overlapping copy of the image (B shifted
    down 4 rows), so PSUM partition p maps to output row 8t + p//16 exactly
    as a single 128-wide stream would.
    """
    nc = tc.nc
    mdt = _mm_bir_dtype()
    f32 = mybir.dt.float32
    cast_in = MM_DTYPE == "bf16"
    x_t = x.tensor
    out_t = out.tensor

    with (
        tc.tile_pool(name="singles", bufs=1) as singles,
        tc.tile_pool(name="xp", bufs=2) as xp,
        tc.tile_pool(name="op", bufs=2) as op,
        tc.tile_pool(name="pp", bufs=6, space="PSUM") as pp,
    ):
        w_tile = singles.tile([112, 5, 64], mdt)
        nc.sync.dma_start(out=w_tile, in_=w)
        a_tile = singles.tile([128, 1], f32)
        nc.sync.dma_start(out=a_tile, in_=alpha)

        for b in range(B_PER_CORE):
            xt = xp.tile([112, 32, 256], mdt)
            # Stream A: partitions r*6+c (r in [0,8)), chunk t <- row 8t+r:
            # covers rows 0..255 exactly once; (c,t) merges (32*8 rows = the
            # 256-row channel stride).
            src_a = bass.AP(
                tensor=x_t, offset=b * IMG,
                ap=[[W, 8], [8 * W, 6 * 32], [1, W]])
            # Stream B: same structure shifted 4 rows down (rows 4..259;
            # the last 4 rows of channel 5 land in the zero tail padding /
            # next image and only feed never-written output rows).
            src_b = bass.AP(
                tensor=x_t, offset=b * IMG + 4 * W,
                ap=[[W, 8], [8 * W, 6 * 32], [1, W]])
            # gpsimd = SWDGE descriptor ring, so input traffic does not
            # serialize behind the output DMAs on the HWDGE ring
            nc.gpsimd.dma_start(out=xt[0:48, :, :], in_=src_a)
            nc.gpsimd.dma_start(out=xt[64:112, :, :], in_=src_b)

            for h in range(2):
                ot = op.tile([128, 16, WO], f32)
                for u in range(8):
                    t0 = 16 * h + 2 * u
                    ps = pp.tile([128, 2, WO], f32)
                    for kw in range(5):
                        # the two streams accumulate in disjoint partition
                        # halves of one PSUM bank; bass's zero-region group
                        # tracker is tile-granular, so skip it
                        nc.tensor.matmul(
                            ps[0:64, :, :],
                            lhsT=w_tile[0:48, kw, :],
                            rhs=xt[0:48, t0:t0 + 2, kw:kw + WO],
                            start=(kw == 0), stop=(kw == 4),
                            skip_group_check=True)
                        nc.tensor.matmul(
                            ps[64:128, :, :],
                            lhsT=w_tile[64:112, kw, :],
                            rhs=xt[64:112, t0:t0 + 2, kw:kw + WO],
                            start=(kw == 0), stop=(kw == 4),
                            skip_group_check=True)
                    dst = ot[:, 2 * u:2 * u + 2, :]
                    if u % 2 == 0:
                        nc.vector.tensor_scalar_mul(dst, ps, a_tile)
                    else:
                        nc.scalar.activation(
                            dst, ps, mybir.ActivationFunctionType.Copy,
                            scale=a_tile)
                # write back: rows 128h + {8c+i}, one DMA per i (16
                # contiguous partitions each), 15 chunks in h=1 (chunk 31's
                # valid 4 rows go in the tail DMA below)
                nch = 16 if h == 0 else 15
                for i in range(8):
                    dst_ap = bass.AP(
                        tensor=out_t,
                        offset=b * OIMG + (128 * h + i) * WO,
                        ap=[[HO * WO, 16], [8 * WO, nch], [1, WO]])
                    nc.sync.dma_start(out=dst_ap,
                                      in_=ot[i * 16:(i + 1) * 16, 0:nch, :])
                if h == 1:
                    # out rows 248..251 (chunk 31, i in [0,4))
                    dst_ap = bass.AP(
                        tensor=out_t, offset=b * OIMG + 248 * WO,
                        ap=[[WO, 4], [HO * WO, 16], [1, WO]])
                    nc.sync.dma_start(out=dst_ap, in_=ot[0:64, 15, :])


def _build_nc():
    nc = bacc.Bacc("TRN2", target_bir_lowering=False, debug=False,
                   num_devices=N_CORES)
    f32 = mybir.dt.float32
    # bf16 mode casts during the (SWDGE) input DMA, so dram x stays f32;
    # fp32/fp32r modes DMA via HWDGE with no cast, so dtypes must match.
    x_dt = f32 if MM_DTYPE == "bf16" else _mm_bir_dtype()
    x = nc.dram_tensor("x", [B_PER_CORE * IMG + XPAD], x_dt,
                       kind="ExternalInput").ap()
    w = nc.dram_tensor("w", [112, 5, 64], _mm_bir_dtype(),
                       kind="ExternalInput").ap()
    alpha = nc.dram_tensor("alpha", [128, 1], f32, kind="ExternalInput").ap()
    out = nc.dram_tensor("out", [B_PER_CORE, 16, HO, WO], f32,
                         kind="ExternalOutput").ap()
    with tile.TileContext(nc) as tc:
        _emit(tc, x, w, alpha, out)
    nc.compile()
    return nc


def _get_nc():
    if "nc" not in _CACHED:
        _CACHED["nc"] = _build_nc()
    return _CACHED["nc"]


def _make_in_maps(input, W3, W4, W6):
    sign_dense, alpha16 = _pack_weights(W3, W4, W6)
    lhsT = _build_lhsT(sign_dense)                       # [5, 48, 64]
    w_host = np.zeros((112, 5, 64), np.float32)
    w_host[0:48] = lhsT.transpose(1, 0, 2)               # stream A
    w_host[64:112] = lhsT.transpose(1, 0, 2)             # stream B
    w_host = w_host.astype(_mm_np_dtype())
    alpha_host = np.tile(alpha16, 8).reshape(128, 1).astype(np.float32)
    inp = np.asarray(input, np.float32)
    pad = np.zeros(XPAD, np.float32)
    in_maps = []
    for c in range(N_CORES):
        xc = inp[c * B_PER_CORE:(c + 1) * B_PER_CORE].ravel()
        in_maps.append({
            "x": np.concatenate([xc, pad]),
            "w": w_host,
            "alpha": alpha_host,
        })
    return in_maps


def run(input, W3, W4, W6, trace=False):
    """Run the kernel; returns (full_output, BassKernelResults)."""
    nc = _get_nc()
    in_maps = _make_in_maps(input, W3, W4, W6)
    res = bass_utils.run_bass_kernel_spmd(
        nc, in_maps, core_ids=list(range(N_CORES)), trace=trace)
    out = np.concatenate([np.asarray(r["out"]) for r in res.results], axis=0)
    return out, res


def kernel(input, W3, W4, W6):
    out, _ = run(input, W3, W4, W6)
    return out


# revision 15
# speedup vs baseline: 1.7263x; 1.1390x over previous
"""Trainium2 Bass kernel for the LeNet-C3-style binarized 16-branch 5x5 conv.

Problem: input [64, 6, 256, 256] f32 -> output [64, 16, 252, 252] f32.
Each of 16 branches g convolves a channel subset S_g (classic LeNet C3
connectivity) with a binarized 5x5 kernel whose taps are all +-alpha_g.

Strategy (pure data parallel over 8 NeuronCores, 8 images per core):
  - Host: binarize weights -> sign tensor (+-1/0, exact in bf16) and per-branch
    scale alpha_g (folded into the PSUM evacuation, so the matmul itself
    carries no weight quantization error).
  - Device, per image: view the image as 32 row-chunks of 8 rows. SBUF X-tile
    [72, 32, 256]: partition (r*6+c), chunk t holds input row 8t+r of channel
    c (12-row window = 8 output rows + 4 halo; 1.5x HBM read amplification).
    For each pair of chunks, 5 accumulating matmuls (one per kw tap, rhs
    shifted by kw in the width dim) against a stationary [72, 128] sign matrix
    produce PSUM[(i*16+g), (t,j)] = conv for 16 output rows x 16 channels x
    252 columns. DVE/ACT evacuate PSUM -> SBUF applying the per-partition
    alpha_g scale; strided DMAs write half-images back to HBM.
"""

import numpy as np
import ml_dtypes

import concourse.bass as bass
import concourse.mybir as mybir
import concourse.tile as tile
from concourse import bacc, bass_utils

# LeNet C3 connectivity
FM = [[0, 1, 2], [1, 2, 3], [2, 3, 4], [3, 4, 5], [0, 4, 5], [0, 1, 5],
      [0, 1, 2, 3], [1, 2, 3, 4], [2, 3, 4, 5], [0, 3, 4, 5], [0, 1, 4, 5],
      [0, 1, 2, 5], [0, 1, 3, 4], [1, 2, 4, 5], [0, 2, 3, 5],
      [0, 1, 2, 3, 4, 5]]

N_CORES = 8
B_PER_CORE = 8
H = W = 256
HO = WO = 252
IMG = 6 * H * W            # 393216 elements per input image
OIMG = 16 * HO * WO        # 1016064 elements per output image
XPAD = 1024                # flat-input tail padding (chunk-31 halo overrun)
# moving/stationary matmul dtype: "bf16" (1 cyc/row, input rounded to bf16),
# "fp32" (exact, 4 cyc/row), "fp32r" (1 cyc/row at N>=256, HW-reduced fp32)
MM_DTYPE = "bf16"

_CACHED = {}


def _binarize_np(w):
    saidas = float(w.shape[1] * w.shape[2] * w.shape[3])
    mw = w - np.mean(w, axis=1, keepdims=True, dtype=np.float32)
    cw = np.clip(mw, -1.0, 1.0)
    alpha = np.sum(np.abs(cw), axis=(1, 2, 3)) / saidas
    return np.sign(cw).astype(np.float32), alpha.astype(np.float32)


def _pack_weights(W3, W4, W6):
    """sign_dense [16,6,5,5] in {-1,0,1}; alpha [16]."""
    sign_dense = np.zeros((16, 6, 5, 5), np.float32)
    alpha = np.zeros(16, np.float32)
    for stack, base in ((W3, 0), (W4, 6), (W6, 15)):
        s, a = _binarize_np(np.asarray(stack, np.float32))
        for gi in range(s.shape[0]):
            g = base + gi
            for ci, c in enumerate(FM[g]):
                sign_dense[g, c] = s[gi, ci]
            alpha[g] = a[gi]
    return sign_dense, alpha


def _build_lhsT(sign_dense):
    """lhsT [5(kw), 48(r*6+c; r in [0,8)), 64(i*16+g; i in [0,4))].

    One 4-output-row stream; the same matrix serves both column-tiled
    streams (stream B is the identical structure shifted 4 rows down).
    """
    lhsT = np.zeros((5, 48, 64), np.float32)
    for kw in range(5):
        for i in range(4):
            for kh in range(5):
                r = i + kh
                for c in range(6):
                    lhsT[kw, r * 6 + c, i * 16:(i + 1) * 16] = \
                        sign_dense[:, c, kh, kw]
    return lhsT


def _mm_np_dtype():
    return {"bf16": ml_dtypes.bfloat16,
            "fp32": np.float32,
            "fp32r": np.float32}[MM_DTYPE]


def _mm_bir_dtype():
    return {"bf16": mybir.dt.bfloat16,
            "fp32": mybir.dt.float32,
            "fp32r": mybir.dt.float32r}[MM_DTYPE]


def _emit(tc, x, w, alpha, out):
    """Emit the per-core program.

    x:     dram [B_PER_CORE*IMG + XPAD] mm-dtype (flat, zero-padded)
    w:     dram [112, 5, 64] mm-dtype (sign weights per kw tap; the same
           [48, 64] matrix at partition 0 (stream A) and 64 (stream B))
    alpha: dram [128, 1] f32 (alpha[g] tiled over i)
    out:   dram [B_PER_CORE, 16, 252, 252] f32

    Two column-tiled matmul streams run concurrently in disjoint PE
    quadrants: stream A (array rows 0-47, cols 0-63) computes output rows
    8t..8t+3; stream B (rows 64-111, cols 64-127) rows 8t+4..8t+7. Each
    stream's X block is a full non-overlapping copy of the image (B shifted
    down 4 rows), so PSUM partition p maps to output row 8t + p//16 exactly
    as a single 128-wide stream would.
    """
    nc = tc.nc
    mdt = _mm_bir_dtype()
    f32 = mybir.dt.float32
    cast_in = MM_DTYPE == "bf16"
    x_t = x.tensor
    out_t = out.tensor

    with (
        tc.tile_pool(name="singles", bufs=1) as singles,
        tc.tile_pool(name="xp", bufs=3) as xp,
        tc.tile_pool(name="op", bufs=2) as op,
        tc.tile_pool(name="pp", bufs=8, space="PSUM") as pp,
    ):
        w_tile = singles.tile([112, 5, 64], mdt)
        nc.sync.dma_start(out=w_tile, in_=w)
        a_tile = singles.tile([128, 1], f32)
        nc.sync.dma_start(out=a_tile, in_=alpha)

        for b in range(B_PER_CORE):
            xt = xp.tile([112, 32, 256], mdt)
            # Stream A: partitions r*6+c (r in [0,8)), chunk t <- row 8t+r:
            # covers rows 0..255 exactly once; (c,t) merges (32*8 rows = the
            # 256-row channel stride).
            src_a = bass.AP(
                tensor=x_t, offset=b * IMG,
                ap=[[W, 8], [8 * W, 6 * 32], [1, W]])
            # Stream B: same structure shifted 4 rows down (rows 4..259;
            # the last 4 rows of channel 5 land in the zero tail padding /
            # next image and only feed never-written output rows).
            src_b = bass.AP(
                tensor=x_t, offset=b * IMG + 4 * W,
                ap=[[W, 8], [8 * W, 6 * 32], [1, W]])
            # gpsimd = SWDGE descriptor ring, so input traffic does not
            # serialize behind the output DMAs on the HWDGE ring
            nc.gpsimd.dma_start(out=xt[0:48, :, :], in_=src_a)
            nc.gpsimd.dma_start(out=xt[64:112, :, :], in_=src_b)

            for h in range(2):
                ot = op.tile([128, 16, WO], f32)
                for u in range(8):
                    t0 = 16 * h + 2 * u
                    ps = pp.tile([128, 2, WO], f32)
                    for kw in range(5):
                        # the two streams accumulate in disjoint partition
                        # halves of one PSUM bank; bass's zero-region group
                        # tracker is tile-granular, so skip it
                        nc.tensor.matmul(
                            ps[0:64, :, :],
                            lhsT=w_tile[0:48, kw, :],
                            rhs=xt[0:48, t0:t0 + 2, kw:kw + WO],
                            start=(kw == 0), stop=(kw == 4),
                            skip_group_check=True)
                        nc.tensor.matmul(
                            ps[64:128, :, :],
                            lhsT=w_tile[64:112, kw, :],
                            rhs=xt[64:112, t0:t0 + 2, kw:kw + WO],
                            start=(kw == 0), stop=(kw == 4),
                            skip_group_check=True)
                    dst = ot[:, 2 * u:2 * u + 2, :]
                    if u % 2 == 0:
                        nc.vector.tensor_scalar_mul(dst, ps, a_tile)
                    else:
                        nc.scalar.activation(
                            dst, ps, mybir.ActivationFunctionType.Copy,
                            scale=a_tile)
                # write back: rows 128h + {8c+i}, one DMA per i (16
                # contiguous partitions each), 15 chunks in h=1 (chunk 31's
                # valid 4 rows go in the tail DMA below)
                nch = 16 if h == 0 else 15
                last_half = (b == B_PER_CORE - 1 and h == 1)
                for i in range(8):
                    # alternate the two HWDGE rings (SP / ACT sequencer) so
                    # output descriptors feed from two FIFOs
                    eng = nc.sync if i % 2 == 0 else nc.scalar
                    if last_half:
                        # split by chunk-halves so the final writes overlap
                        # the remaining evacuations (shrinks the kernel tail)
                        for c0, c1 in ((0, 8), (8, nch)):
                            dst_ap = bass.AP(
                                tensor=out_t,
                                offset=b * OIMG + (128 * h + 8 * c0 + i) * WO,
                                ap=[[HO * WO, 16], [8 * WO, c1 - c0], [1, WO]])
                            eng.dma_start(
                                out=dst_ap,
                                in_=ot[i * 16:(i + 1) * 16, c0:c1, :])
                    else:
                        dst_ap = bass.AP(
                            tensor=out_t,
                            offset=b * OIMG + (128 * h + i) * WO,
                            ap=[[HO * WO, 16], [8 * WO, nch], [1, WO]])
                        eng.dma_start(out=dst_ap,
                                      in_=ot[i * 16:(i + 1) * 16, 0:nch, :])
                if h == 1:
                    # out rows 248..251 (chunk 31, i in [0,4))
                    dst_ap = bass.AP(
                        tensor=out_t, offset=b * OIMG + 248 * WO,
                        ap=[[WO, 4], [HO * WO, 16], [1, WO]])
                    nc.sync.dma_start(out=dst_ap, in_=ot[0:64, 15, :])


def _build_nc():
    nc = bacc.Bacc("TRN2", target_bir_lowering=False, debug=False,
                   num_devices=N_CORES)
    f32 = mybir.dt.float32
    # bf16 mode casts during the (SWDGE) input DMA, so dram x stays f32;
    # fp32/fp32r modes DMA via HWDGE with no cast, so dtypes must match.
    x_dt = f32 if MM_DTYPE == "bf16" else _mm_bir_dtype()
    x = nc.dram_tensor("x", [B_PER_CORE * IMG + XPAD], x_dt,
                       kind="ExternalInput").ap()
    w = nc.dram_tensor("w", [112, 5, 64], _mm_bir_dtype(),
                       kind="ExternalInput").ap()
    alpha = nc.dram_tensor("alpha", [128, 1], f32, kind="ExternalInput").ap()
    out = nc.dram_tensor("out", [B_PER_CORE, 16, HO, WO], f32,
                         kind="ExternalOutput").ap()
    with tile.TileContext(nc) as tc:
        _emit(tc, x, w, alpha, out)
    nc.compile()
    return nc


def _get_nc():
    if "nc" not in _CACHED:
        _CACHED["nc"] = _build_nc()
    return _CACHED["nc"]


def _make_in_maps(input, W3, W4, W6):
    sign_dense, alpha16 = _pack_weights(W3, W4, W6)
    lhsT = _build_lhsT(sign_dense)                       # [5, 48, 64]
    w_host = np.zeros((112, 5, 64), np.float32)
    w_host[0:48] = lhsT.transpose(1, 0, 2)               # stream A
    w_host[64:112] = lhsT.transpose(1, 0, 2)             # stream B
    w_host = w_host.astype(_mm_np_dtype())
    alpha_host = np.tile(alpha16, 8).reshape(128, 1).astype(np.float32)
    inp = np.asarray(input, np.float32)
    pad = np.zeros(XPAD, np.float32)
    in_maps = []
    for c in range(N_CORES):
        xc = inp[c * B_PER_CORE:(c + 1) * B_PER_CORE].ravel()
        in_maps.append({
            "x": np.concatenate([xc, pad]),
            "w": w_host,
            "alpha": alpha_host,
        })
    return in_maps


def run(input, W3, W4, W6, trace=False):
    """Run the kernel; returns (full_output, BassKernelResults)."""
    nc = _get_nc()
    in_maps = _make_in_maps(input, W3, W4, W6)
    res = bass_utils.run_bass_kernel_spmd(
        nc, in_maps, core_ids=list(range(N_CORES)), trace=trace)
    out = np.concatenate([np.asarray(r["out"]) for r in res.results], axis=0)
    return out, res


def kernel(input, W3, W4, W6):
    out, _ = run(input, W3, W4, W6)
    return out
